# revision 2
# baseline (speedup 1.0000x reference)
"""Stacked tanh-RNN (4 layers, B=32, T=2048, NIN=256, H=512) on 8 trn2 cores.

Sharding: data-parallel over batch (4 sequences per core). Everything on
the device is kept feature-major ("transposed"): activations are stored as
[feature, column] with column = t*B_loc + b. This makes the weights the
stationary matmul operand everywhere and the recurrence h_{t} = tanh(xp_t +
W_hh h_{t-1}) runs without any transposes:

  - layer GEMM:  xp^T[ho, col] = W^T.T @ in^T  (PE, k-chunk accumulated)
  - scan step:   psum[ho, b]  += Whh^T[k].T @ h^T[k, b]   (16 MMs)
                 tmp = psum + xp[:, t]  (DVE)   h = tanh(tmp)  (ACT)

Host side pre-transposes x and the weights, post-transposes the output.
"""

import numpy as np

import concourse.bacc as bacc
import concourse.mybir as mybir
import concourse.tile as tile
from concourse.bass_utils import run_bass_kernel_spmd

B, T, NIN, H, L = 32, 2048, 256, 512, 4
NCORES = 8
BLOC = B // NCORES          # sequences per core
FP = mybir.dt.float32
ACTF = mybir.ActivationFunctionType

_built = {}


def build(T_=T, chunk=512, blk=128):
    """Build + compile the per-core Bass program (SPMD identical)."""
    cols = T_ * BLOC
    chunk = min(chunk, cols)
    blk = min(blk, T_)
    nc = bacc.Bacc(None, target_bir_lowering=False)

    xT = nc.dram_tensor("xT", [NIN, cols], FP, kind="ExternalInput")
    wih0T = nc.dram_tensor("wih0T", [NIN, H], FP, kind="ExternalInput")
    wihT = nc.dram_tensor("wihT", [L - 1, H, H], FP, kind="ExternalInput")
    whhT = nc.dram_tensor("whhT", [L, H, H], FP, kind="ExternalInput")
    bias_d = nc.dram_tensor("bias", [L, H], FP, kind="ExternalInput")
    outT = nc.dram_tensor("outT", [H, cols], FP, kind="ExternalOutput")
    hbuf = [nc.dram_tensor(f"hbuf{l}", [H, cols], FP) for l in range(L - 1)]

    MH = H // 128  # output-feature chunks (4)

    with tile.TileContext(nc) as tc:
        with (
            tc.tile_pool(name="xp", bufs=1) as xppool,
            tc.tile_pool(name="wgt", bufs=1) as wgt,
            tc.tile_pool(name="bia", bufs=2) as biapool,
            tc.tile_pool(name="inp", bufs=2) as inpool,
            tc.tile_pool(name="stg", bufs=2) as stpool,
            tc.tile_pool(name="tmp", bufs=3) as tmpool,
            tc.tile_pool(name="psg", bufs=2, space="PSUM") as psg,
            tc.tile_pool(name="pss", bufs=4, space="PSUM") as pss,
        ):
            for l in range(L):
                din = NIN if l == 0 else H
                kin = din // 128
                in_dram = xT if l == 0 else hbuf[l - 1]
                out_dram = outT if l == L - 1 else hbuf[l]
                wT_dram = wih0T if l == 0 else wihT[l - 1]

                wih_sb = []
                for k in range(kin):
                    wtile = wgt.tile([128, H], FP, tag=f"wih{k}")
                    nc.sync.dma_start(
                        out=wtile, in_=wT_dram[k * 128:(k + 1) * 128, :]
                    )
                    wih_sb.append(wtile)
                whh_sb = []
                for k in range(MH):
                    wtile = wgt.tile([128, H], FP, tag=f"whh{k}")
                    nc.sync.dma_start(
                        out=wtile, in_=whhT[l, k * 128:(k + 1) * 128, :]
                    )
                    whh_sb.append(wtile)
                bias_sb = []
                for m in range(MH):
                    btile = biapool.tile([128, 1], FP, tag=f"b{m}")
                    nc.sync.dma_start(
                        out=btile,
                        in_=bias_d[l, m * 128:(m + 1) * 128].rearrange(
                            "(p o) -> p o", o=1
                        ),
                    )
                    bias_sb.append(btile)

                # xp[p, m, col] = input-projection + bias for ho = m*128+p
                xp = xppool.tile([128, MH, cols], FP, tag="xp")

                # ---- GEMM phase: xp = W_in^T.T @ in + bias ----
                for c in range(cols // chunk):
                    rhs_k = []
                    for k in range(kin):
                        t_in = inpool.tile([128, chunk], FP, tag=f"rhs{k}")
                        nc.sync.dma_start(
                            out=t_in,
                            in_=in_dram[
                                k * 128:(k + 1) * 128,
                                c * chunk:(c + 1) * chunk,
                            ],
                        )
                        rhs_k.append(t_in)
                    for m in range(MH):
                        ps = psg.tile([128, chunk], FP, tag="psg")
                        for k in range(kin):
                            nc.tensor.matmul(
                                ps,
                                wih_sb[k][:, m * 128:(m + 1) * 128],
                                rhs_k[k],
                                start=(k == 0),
                                stop=(k == kin - 1),
                            )
                        nc.scalar.activation(
                            xp[:, m, c * chunk:(c + 1) * chunk],
                            ps,
                            ACTF.Identity,
                            bias=bias_sb[m],
                        )

                # ---- scan phase ----
                prev_stag = None
                for b0 in range(T_ // blk):
                    stag = stpool.tile([128, MH, blk * BLOC], FP, tag="stag")
                    for tt in range(blk):
                        t = b0 * blk + tt
                        if t == 0:
                            nc.scalar.activation(
                                stag[:, :, 0:BLOC],
                                xp[:, :, 0:BLOC],
                                ACTF.Tanh,
                            )
                            continue
                        if tt == 0:
                            rtile, rcol = prev_stag, (blk - 1) * BLOC
                        else:
                            rtile, rcol = stag, (tt - 1) * BLOC
                        ps = pss.tile([128, MH, BLOC], FP, tag="pss")
                        nmm = 0
                        for k in range(MH):
                            for m in range(MH):
                                nmm += 1
                                nc.tensor.matmul(
                                    ps[:, m, :],
                                    whh_sb[k][:, m * 128:(m + 1) * 128],
                                    rtile[:, k, rcol:rcol + BLOC],
                                    start=(nmm == 1),
                                    stop=(nmm == MH * MH),
                                    skip_group_check=True,
                                )
                        tmp = tmpool.tile([128, MH, BLOC], FP, tag="tmp")
                        nc.vector.tensor_add(
                            tmp, ps, xp[:, :, t * BLOC:(t + 1) * BLOC]
                        )
                        nc.scalar.activation(
                            stag[:, :, tt * BLOC:(tt + 1) * BLOC],
                            tmp,
                            ACTF.Tanh,
                        )
                    for m in range(MH):
                        nc.sync.dma_start(
                            out=out_dram[
                                m * 128:(m + 1) * 128,
                                b0 * blk * BLOC:(b0 + 1) * blk * BLOC,
                            ],
                            in_=stag[:, m, :],
                        )
                    prev_stag = stag

    nc.compile()
    return nc


def _prep_inputs(x, W_ih0, W_ih, W_hh, b_ih, b_hh, T_=T):
    wih0T = np.ascontiguousarray(W_ih0.T.astype(np.float32))
    wihT = np.ascontiguousarray(W_ih.transpose(0, 2, 1).astype(np.float32))
    whhT = np.ascontiguousarray(W_hh.transpose(0, 2, 1).astype(np.float32))
    bias = (b_ih + b_hh).astype(np.float32)
    maps = []
    for c in range(NCORES):
        xc = x[c * BLOC:(c + 1) * BLOC, :T_]  # [BLOC, T_, NIN]
        xTc = np.ascontiguousarray(
            xc.transpose(2, 1, 0).reshape(NIN, T_ * BLOC).astype(np.float32)
        )
        maps.append(
            {"xT": xTc, "wih0T": wih0T, "wihT": wihT, "whhT": whhT,
             "bias": bias}
        )
    return maps


def _post(results, T_=T):
    outs = []
    for c in range(NCORES):
        oT = results[c]["outT"]  # [H, T_*BLOC]
        outs.append(oT.reshape(H, T_, BLOC).transpose(2, 1, 0))
    return np.ascontiguousarray(np.concatenate(outs, 0), dtype=np.float32)


def kernel(x, W_ih0, W_ih, W_hh, b_ih, b_hh, trace=False):
    key = T
    if key not in _built:
        _built[key] = build(T)
    nc = _built[key]
    in_maps = _prep_inputs(x, W_ih0, W_ih, W_hh, b_ih, b_hh)
    res = run_bass_kernel_spmd(nc, in_maps, list(range(NCORES)), trace=trace)
    out = _post(res.results)
    kernel.last_exec_time_ns = res.exec_time_ns
    kernel.last_results = res
    return out


# revision 3
# speedup vs baseline: 4.2293x; 4.2293x over previous
"""Stacked tanh-RNN (B=32, T=2048, NIN=256, H=512, L=4) on 8 trn2 cores.

Sharding: data-parallel over batch (4 sequences per core; hint followed).
Per core, everything is feature-major ([feature, col] with col = t*4 + b),
making weights the stationary matmul operand everywhere and the recurrence
transpose-free. In-core layer wavefront: The four layers' (GEMM + scan) are emitted
chunk-by-chunk in wavefront order, so layer l's scan of chunk c overlaps
layer l-1's scan of chunk c+1 on the engines — the per-step chain latency
(PE→DVE→ACT→PE) hides behind other layers' matmul bursts. Inter-layer
activations stay in SBUF (stag tiles feed the next layer's GEMM directly);
only x comes in and the last layer goes out to DRAM. All matmuls fp32r
(measured ~25ns/MM vs fp32's ~260ns; end-to-end rel err ~2e-3)."""

import numpy as np

import concourse.bacc as bacc
import concourse.mybir as mybir
import concourse.tile as tile
from concourse.bass_utils import run_bass_kernel_spmd

B, T, NIN, H, L = 32, 2048, 256, 512, 4
NCORES = 8
BLOC = B // NCORES
FP = mybir.dt.float32
FPR = mybir.dt.float32r
ACTF = mybir.ActivationFunctionType

_built = {}


def build(T_=T, ck=64, rep=1):
    """ck = timesteps per wavefront chunk; rep repeats the whole net (timing)."""
    ck = min(ck, T_)
    cols = T_ * BLOC
    ckc = ck * BLOC          # columns per chunk
    C = T_ // ck             # number of chunks
    MH = H // 128

    nc = bacc.Bacc(None, target_bir_lowering=False)

    xT = nc.dram_tensor("xT", [NIN, cols], FPR, kind="ExternalInput")
    wih0T = nc.dram_tensor("wih0T", [NIN, H], FPR, kind="ExternalInput")
    wihT = nc.dram_tensor("wihT", [L - 1, H, H], FPR, kind="ExternalInput")
    whhT = nc.dram_tensor("whhT", [L, H, H], FPR, kind="ExternalInput")
    bias_d = nc.dram_tensor("bias", [L, H], FP, kind="ExternalInput")
    outT = nc.dram_tensor("outT", [H, cols], FPR, kind="ExternalOutput")

    with tile.TileContext(nc) as tc:
        with (
            tc.tile_pool(name="wgt", bufs=1) as wgt,
            tc.tile_pool(name="bia", bufs=1) as biapool,
            tc.tile_pool(name="inp", bufs=3) as inpool,
            tc.tile_pool(name="xpp", bufs=2) as xppool,
            tc.tile_pool(name="stg", bufs=2) as stpool,
            tc.tile_pool(name="tmp", bufs=4) as tmpool,
            tc.tile_pool(name="psg", bufs=2, space="PSUM") as psg,
            tc.tile_pool(name="pss", bufs=4, space="PSUM") as pss,
        ):
            # --- load all weights/biases once ---
            wih_sb, whh_sb, bias_sb = [], [], []
            for l in range(L):
                kin = (NIN if l == 0 else H) // 128
                wT_dram = wih0T if l == 0 else wihT[l - 1]
                wl = []
                for k in range(kin):
                    wtile = wgt.tile([128, H], FPR, tag=f"wih{l}_{k}",
                                     name=f"wih{l}_{k}")
                    nc.sync.dma_start(
                        out=wtile, in_=wT_dram[k * 128:(k + 1) * 128, :])
                    wl.append(wtile)
                wih_sb.append(wl)
                hl = []
                for k in range(MH):
                    wtile = wgt.tile([128, H], FPR, tag=f"whh{l}_{k}",
                                     name=f"whh{l}_{k}")
                    nc.sync.dma_start(
                        out=wtile, in_=whhT[l, k * 128:(k + 1) * 128, :])
                    hl.append(wtile)
                whh_sb.append(hl)
                bl = []
                for m in range(MH):
                    btile = biapool.tile([128, 1], FP, tag=f"b{l}_{m}",
                                         name=f"b{l}_{m}")
                    nc.sync.dma_start(
                        out=btile,
                        in_=bias_d[l, m * 128:(m + 1) * 128].rearrange(
                            "(p o) -> p o", o=1))
                    bl.append(btile)
                bias_sb.append(bl)

            for _rep in range(rep):
             # per-layer rolling state
             prev_stag = [None] * L
             for c in range(C):
                c0 = c * ckc
                for l in range(L):
                    kin = (NIN if l == 0 else H) // 128

                    # ---- GEMM for (l, c): xp = W^T.T @ in + bias ----
                    if l == 0:
                        rhs_k = []
                        for k in range(kin):
                            t_in = inpool.tile([128, ckc], FPR, tag=f"rhs{k}",
                                               name=f"rhs{k}_{_rep}_{c}")
                            nc.sync.dma_start(
                                out=t_in,
                                in_=xT[k * 128:(k + 1) * 128, c0:c0 + ckc])
                            rhs_k.append(t_in)
                        rhs = lambda k: rhs_k[k]
                    else:
                        src = stag_done[l - 1]  # this chunk's output of l-1
                        rhs = lambda k: src[:, k, :]

                    xp = xppool.tile([128, MH, ckc], FP, tag=f"xp{l}",
                                     name=f"xp{l}_{_rep}_{c}")
                    for m in range(MH):
                        ps = psg.tile([128, ckc], FP, tag="psg",
                                      name=f"psg{l}_{_rep}_{c}_{m}")
                        for k in range(kin):
                            nc.tensor.matmul(
                                ps, wih_sb[l][k][:, m * 128:(m + 1) * 128],
                                rhs(k), start=(k == 0), stop=(k == kin - 1))
                        nc.scalar.activation(
                            xp[:, m, :], ps, ACTF.Identity, bias=bias_sb[l][m])

                    # ---- scan for (l, c) ----
                    stag = stpool.tile([128, MH, ckc], FPR, tag=f"stag{l}",
                                       name=f"stag{l}_{_rep}_{c}")
                    for tt in range(ck):
                        t = c0 // BLOC + tt
                        if t == 0:
                            nc.scalar.activation(
                                stag[:, :, 0:BLOC], xp[:, :, 0:BLOC],
                                ACTF.Tanh)
                            continue
                        if tt == 0:
                            rtile, rcol = prev_stag[l], (ck - 1) * BLOC
                        else:
                            rtile, rcol = stag, (tt - 1) * BLOC
                        ps = pss.tile([128, MH, BLOC], FP, tag="pss",
                                      name=f"pss{l}_{_rep}_{c}_{tt}")
                        nmm = 0
                        for m in range(MH):
                            for k in range(MH):
                                nmm += 1
                                nc.tensor.matmul(
                                    ps[:, m, :],
                                    whh_sb[l][k][:, m * 128:(m + 1) * 128],
                                    rtile[:, k, rcol:rcol + BLOC],
                                    start=(nmm == 1), stop=(nmm == MH * MH),
                                    skip_group_check=True)
                        tmp = tmpool.tile([128, MH, BLOC], FP, tag="tmp",
                                          name=f"tmp{l}_{_rep}_{c}_{tt}")
                        nc.vector.tensor_add(
                            tmp, ps, xp[:, :, tt * BLOC:(tt + 1) * BLOC])
                        nc.scalar.activation(
                            stag[:, :, tt * BLOC:(tt + 1) * BLOC], tmp,
                            ACTF.Tanh)

                    if l == L - 1:
                        for m in range(MH):
                            nc.sync.dma_start(
                                out=outT[m * 128:(m + 1) * 128, c0:c0 + ckc],
                                in_=stag[:, m, :])
                    prev_stag[l] = stag
                    if l == 0:
                        stag_done = {}
                    stag_done[l] = stag

    nc.compile()
    return nc


def _prep_inputs(x, W_ih0, W_ih, W_hh, b_ih, b_hh, T_=T):
    wih0T = np.ascontiguousarray(W_ih0.T.astype(np.float32))
    wihT = np.ascontiguousarray(W_ih.transpose(0, 2, 1).astype(np.float32))
    whhT = np.ascontiguousarray(W_hh.transpose(0, 2, 1).astype(np.float32))
    bias = (b_ih + b_hh).astype(np.float32)
    maps = []
    for c in range(NCORES):
        xc = x[c * BLOC:(c + 1) * BLOC, :T_]
        xTc = np.ascontiguousarray(
            xc.transpose(2, 1, 0).reshape(NIN, T_ * BLOC).astype(np.float32))
        maps.append({"xT": xTc, "wih0T": wih0T, "wihT": wihT, "whhT": whhT,
                     "bias": bias})
    return maps


def _post(results, T_=T):
    outs = []
    for c in range(NCORES):
        oT = results[c]["outT"]
        outs.append(oT.reshape(H, T_, BLOC).transpose(2, 1, 0))
    return np.ascontiguousarray(np.concatenate(outs, 0), dtype=np.float32)


def kernel(x, W_ih0, W_ih, W_hh, b_ih, b_hh, trace=False):
    key = T
    if key not in _built:
        _built[key] = build(T)
    nc = _built[key]
    in_maps = _prep_inputs(x, W_ih0, W_ih, W_hh, b_ih, b_hh)
    res = run_bass_kernel_spmd(nc, in_maps, list(range(NCORES)), trace=trace)
    out = _post(res.results)
    kernel.last_exec_time_ns = res.exec_time_ns
    kernel.last_results = res
    return out
